# revision 16
# baseline (speedup 1.0000x reference)
"""H2GCN forward pass distributed over 8 TRN2 NeuronCores.

Sharding: nodes row-sharded across cores; edges partitioned by src owner so
the segment_sum is core-local; r_prev all-gathered between layers; weights
replicated.

v3 design:
  - Gathers use the dedicated DMAGatherAnt instruction (nc.gpsimd.dma_gather):
    int16 indices wrapped over 16 partitions (replicated x8), 256B rows ->
    f32 table [N, 64]. Index range is handled with 4 parity buckets b = dst&3,
    each gathering through the strided view table[b::4] with idx = dst>>2
    (max 24999 < 2^15). One gather instruction per (8-window chunk, bucket).
  - Per (window, bucket) the slots are padded to whole 128-slot tiles with
    cross-core-uniform tile counts RT[w][b]; pad slots point at view row 0
    with weight 0 and nl=-1.
  - Slot tiles exist in two orders over the same runs: gather order
    (bucket-major within chunk; ws + idx arrays) and window-major order
    (nl array + S tiles). A run is a contiguous identical block in both, so
    window w's j-th S tile pairs with gather tile gtiles[w][j].
  - Messages are converted f32->bf16 by the per-region ws multiply on DVE,
    so PE matmuls stay bf16 (1 cycle/row vs 4 for f32).
  - r^T representations live in two resident SBUF tiles (bf16): layer 0 reads
    rA(=r0) and writes rB(=r1); layer 1 reads rB and writes rA(=r2). Fused
    logits in layer 1 read r0's columns BEFORE the r2 activation overwrites
    them (PSUM accumulation reorder).
  - msg/staging buffers are memset once at first use: stale SBUF bytes must be
    finite because NaN*0 = NaN would poison the matmul.
"""

import os
import sys

sys.path.insert(0, "/opt/trn_rl_repo")

import numpy as np
import ml_dtypes

import concourse.bacc as bacc
import concourse.bass as bass
import concourse.mybir as mybir
import concourse.tile as tile
from concourse.bass_utils import run_bass_kernel_spmd
from concourse.masks import make_identity

F32 = mybir.dt.float32
BF16 = mybir.dt.bfloat16
I16 = mybir.dt.int16
AF = mybir.ActivationFunctionType
BF = ml_dtypes.bfloat16
NB = 4  # parity buckets (dst & 3)
NW_CH = 6  # windows per gather chunk
CH = 7  # phase-A windows per x-chunk load


def _prep_set(src, dst, n_nodes, ncores):
    """Bucketed slot arrays for one edge set.

    Returns (per_core list, meta) where per-core dict has:
      idxw: [128, T*8] int16 wrapped gather indices (gather order)
      wsg:  [128, T] bf16 per-slot weights (gather order)
      nlw:  [128, T] bf16 slot->node-local map (window-major order)
    and meta has RT[w][b], tpw[w], gtiles[w], chunk table, T, tpwmax.
    """
    npc = n_nodes // ncores
    nw = (npc + 127) // 128
    deg = np.bincount(src, minlength=n_nodes)
    wglob = (1.0 / np.maximum(deg, 1.0)).astype(np.float32)

    # per-core edge lists sorted by (local node, bucket)
    cores = []
    cnt = np.zeros((ncores, nw, NB), np.int64)
    for c in range(ncores):
        lo = c * npc
        m = (src >= lo) & (src < lo + npc)
        sl = (src[m] - lo).astype(np.int64)
        d = dst[m].astype(np.int64)
        wv = wglob[src[m]]
        b = d & 3
        w = sl // 128
        order = np.lexsort((sl, b, w))  # sort by (window, bucket, node)
        sl, d, wv, b, w = sl[order], d[order], wv[order], b[order], w[order]
        np.add.at(cnt[c], (w, b), 1)
        cores.append((sl, d, wv, b, w))

    RT = np.maximum(1, -(-cnt.max(axis=0) // 128))  # [nw, NB] tiles
    tpw = RT.sum(axis=1)  # [nw]
    tpwmax = int(tpw.max())

    # chunk table: gather-order tile layout (chunk-major, bucket-major)
    chunks = []
    gtile_of = np.zeros((nw, NB), np.int64)  # first gather tile of (w, b)
    wmtile_of = np.zeros((nw, NB), np.int64)  # first window-major tile
    tile_cursor = 0
    for w0 in range(0, nw, NW_CH):
        wn = min(NW_CH, nw - w0)
        regions = []
        for b in range(NB):
            rt = int(RT[w0 : w0 + wn, b].sum())
            regions.append(dict(b=b, tile0=tile_cursor, ntiles=rt))
            for w in range(w0, w0 + wn):
                gtile_of[w, b] = tile_cursor + int(RT[w0:w, b].sum())
            tile_cursor += rt
        chunks.append(dict(w0=w0, wn=wn, regions=regions))
    T = tile_cursor
    cur = 0
    for w in range(nw):
        for b in range(NB):
            wmtile_of[w, b] = cur
            cur += int(RT[w, b])
    gtiles = [
        [
            int(gtile_of[w, b]) * 1 + r
            for b in range(NB)
            for r in range(int(RT[w, b]))
        ]
        for w in range(nw)
    ]
    for ch in chunks:
        ch["tile0"] = ch["regions"][0]["tile0"]
        ch["ntiles"] = sum(r["ntiles"] for r in ch["regions"])

    per_core = []
    for c in range(ncores):
        sl, d, wv, b, w = cores[c]
        idxg = np.zeros(T * 128, np.int16)  # gather order; pad -> view row 0
        wsg = np.zeros(T * 128, np.float32)
        nlw = np.full(T * 128, -1.0, np.float32)  # window-major order
        # rank of each edge within its (w, b) run
        run_start = {}
        pos_g = np.empty(len(sl), np.int64)
        pos_m = np.empty(len(sl), np.int64)
        k = 0
        while k < len(sl):
            w_, b_ = w[k], b[k]
            k2 = k
            while k2 < len(sl) and w[k2] == w_ and b[k2] == b_:
                k2 += 1
            n = k2 - k
            pos_g[k:k2] = gtile_of[w_, b_] * 128 + np.arange(n)
            pos_m[k:k2] = wmtile_of[w_, b_] * 128 + np.arange(n)
            k = k2
        idxg[pos_g] = (d >> 2).astype(np.int16)
        wsg[pos_g] = wv
        nlw[pos_m] = (sl % 128).astype(np.float32)
        # wrap gather idx: position j -> [j % 16, j // 16], replicated x8
        wrap = idxg.reshape(T * 8, 16).T  # [16, T*8]
        idxw = np.tile(wrap, (8, 1))  # [128, T*8]
        per_core.append(
            dict(
                idxw=np.ascontiguousarray(idxw),
                wsg=np.ascontiguousarray(wsg.reshape(T, 128).T.astype(BF)),
                nlw=np.ascontiguousarray(nlw.reshape(T, 128).T.astype(BF)),
            )
        )
    meta = dict(
        RT=RT.tolist(),
        tpw=[int(x) for x in tpw],
        tpwmax=tpwmax,
        gtiles=gtiles,
        chunks=chunks,
        T=T,
    )
    return per_core, meta


def build_program(cfg):
    n_nodes = cfg["n_nodes"]
    npc = cfg["npc"]
    nw = cfg["nw"]
    ncores = cfg["ncores"]
    ipad = cfg["ipad"]
    ncls = cfg["ncls"]
    meta = cfg["meta"]  # [set0, set1]
    KT = ipad // 128
    H = 64
    NWC = nw * 128

    T = [meta[s]["T"] for s in (0, 1)]
    tpwmax = max(meta[0]["tpwmax"], meta[1]["tpwmax"])
    CTmax = max(
        max(ch["ntiles"] for ch in meta[s]["chunks"]) for s in (0, 1)
    )
    RTmax = max(
        max(r["ntiles"] for ch in meta[s]["chunks"] for r in ch["regions"])
        for s in (0, 1)
    )

    MODE = os.environ.get("KBISECT", "full")
    nc = bacc.Bacc(
        "TRN2",
        target_bir_lowering=False,
        debug=False,
        enable_asserts=False,
        num_devices=ncores,
    )

    # --- DRAM I/O ---
    xT = nc.dram_tensor("xT", [128, nw * KT * 128], BF16, kind="ExternalInput")
    wemb = nc.dram_tensor("wemb", [128, KT * H], BF16, kind="ExternalInput")
    bemb = nc.dram_tensor("bemb", [H, 1], F32, kind="ExternalInput")
    wl = [
        nc.dram_tensor(f"wl{i}", [H, 3 * H], BF16, kind="ExternalInput")
        for i in (0, 1)
    ]
    bl = [nc.dram_tensor(f"b{i}", [H, 1], F32, kind="ExternalInput") for i in (0, 1)]
    wc = nc.dram_tensor("wc", [H, 3 * ncls], BF16, kind="ExternalInput")
    bc = nc.dram_tensor("bc", [ncls, 1], F32, kind="ExternalInput")
    iota = nc.dram_tensor("iota", [128, tpwmax, 128], BF16, kind="ExternalInput")
    idx_d = [
        nc.dram_tensor(f"idx{s}", [128, T[s] * 8], I16, kind="ExternalInput")
        for s in (0, 1)
    ]
    ws_d = [
        nc.dram_tensor(f"ws{s}", [128, T[s]], BF16, kind="ExternalInput")
        for s in (0, 1)
    ]
    nl_d = [
        nc.dram_tensor(f"nl{s}", [128, T[s]], BF16, kind="ExternalInput")
        for s in (0, 1)
    ]
    out_d = nc.dram_tensor("logitsT", [ncls, npc], F32, kind="ExternalOutput")

    r_loc = [nc.dram_tensor(f"r{k}_loc", [npc, H], F32) for k in (0, 1)]
    r_tab = [
        nc.dram_tensor(f"r{k}_tab", [n_nodes, H], F32, addr_space="Shared")
        for k in (0, 1)
    ]

    groups = [list(range(ncores))]

    with tile.TileContext(nc) as tc:
        with (
            tc.tile_pool(name="const", bufs=1) as cp,
            tc.tile_pool(name="io", bufs=2) as iop,
            tc.tile_pool(name="msg", bufs=2) as mp,
            tc.tile_pool(name="stg", bufs=2) as sgp,
            tc.tile_pool(name="meta", bufs=2) as mep,
            tc.tile_pool(name="s2", bufs=2) as s2p,
            tc.tile_pool(name="ns", bufs=2) as nsp,
            tc.tile_pool(name="st", bufs=3) as stp,
        ):
            ident = cp.tile([128, 128], BF16, tag="ident")
            make_identity(nc, ident[:])
            iota_sb = cp.tile([128, tpwmax, 128], BF16, tag="iota")
            nc.sync.dma_start(iota_sb[:], iota[:])
            wemb_sb = cp.tile([128, KT, H], BF16, tag="wemb")
            nc.sync.dma_start(
                wemb_sb[:], wemb.ap().rearrange("p (k h) -> p k h", k=KT)
            )
            wl_sb = []
            for i in (0, 1):
                t = cp.tile([H, 3, H], BF16, tag=f"wl{i}")
                nc.sync.dma_start(t[:], wl[i].ap().rearrange("p (s h) -> p s h", s=3))
                wl_sb.append(t)
            wc_sb = cp.tile([H, 3, ncls], BF16, tag="wc")
            nc.sync.dma_start(wc_sb[:], wc.ap().rearrange("p (s h) -> p s h", s=3))
            bemb_sb = cp.tile([H, 1], F32, tag="bemb")
            nc.sync.dma_start(bemb_sb[:], bemb[:])
            bl_sb = []
            for i in (0, 1):
                t = cp.tile([H, 1], F32, tag=f"bl{i}")
                nc.sync.dma_start(t[:], bl[i][:])
                bl_sb.append(t)
            bc_sb = cp.tile([ncls, 1], F32, tag="bc")
            nc.sync.dma_start(bc_sb[:], bc[:])
            nl_sb = []
            ws_sb = []
            for s in (0, 1):
                t = cp.tile([128, T[s]], BF16, tag=f"nl{s}", name=f"nl{s}")
                nc.sync.dma_start(t[:], nl_d[s][:])
                nl_sb.append(t)
                t = cp.tile([128, T[s]], BF16, tag=f"ws{s}", name=f"ws{s}")
                nc.sync.dma_start(t[:], ws_d[s][:])
                ws_sb.append(t)

            # two resident transposed representations [64, NWC] bf16
            rA = cp.tile([H, NWC], BF16, tag="rA")  # r0, later overwritten by r2
            rB = cp.tile([H, NWC], BF16, tag="rB")  # r1

            # ---------------- Phase A: r0 = relu(x @ Wemb + b) ----------------
            with tc.tile_pool(name="psA", bufs=2, space="PSUM") as psA:
                for clo in range(0, nw, CH):
                    chn = min(CH, nw - clo)
                    xw = iop.tile([128, CH, KT, 128], BF16, tag="xw")
                    nc.sync.dma_start(
                        xw[:, :chn],
                        xT.ap().rearrange("p (w k n) -> p w k n", k=KT, n=128)[
                            :, clo : clo + chn
                        ],
                    )
                    for wi in range(chn):
                        w = clo + wi
                        nodes = min(128, npc - w * 128)
                        cols = slice(w * 128, w * 128 + nodes)
                        ps = psA.tile([H, 128], F32, tag="e")
                        for k in range(KT):
                            nc.tensor.matmul(
                                ps[:, :nodes],
                                wemb_sb[:, k, :],
                                xw[:, wi, k, :nodes],
                                start=(k == 0),
                                stop=(k == KT - 1),
                            )
                        nc.scalar.activation(
                            rA[:, cols], ps[:, :nodes], AF.Relu, bias=bemb_sb[:, :1]
                        )
                        pst = psA.tile([128, H], BF16, tag="tr")
                        nc.tensor.transpose(pst[:nodes, :], rA[:, cols], ident[:H, :H])
                        r0_st = stp.tile([128, H], F32, tag="rrow")
                        nc.scalar.activation(r0_st[:nodes, :], pst[:nodes, :], AF.Copy)
                        nc.sync.dma_start(
                            r_loc[0].ap()[w * 128 : w * 128 + nodes, :],
                            r0_st[:nodes, :],
                        )

            nc.gpsimd.collective_compute(
                "AllGather",
                mybir.AluOpType.bypass,
                replica_groups=groups,
                ins=[r_loc[0].ap().opt()],
                outs=[r_tab[0].ap().opt()],
            )

            # ---------------- Layers ----------------
            if MODE == "A":
                zt = cp.tile([ncls, 128], F32, tag="zero")
                nc.gpsimd.memset(zt[:], 0.0)
                for w in range(nw):
                    nodes = min(128, npc - w * 128)
                    nc.sync.dma_start(
                        out_d.ap()[:, w * 128 : w * 128 + nodes], zt[:, :nodes]
                    )
            memset_done = {}
            for li in ((), (0, 1))[MODE != "A"]:
                table = r_tab[li]
                # strided per-bucket views: rows b::4, stride 4*64 elems
                views = [table.ap()[b::4, :] for b in range(NB)]
                r_prev = rA if li == 0 else rB
                r_cur = rB if li == 0 else rA
                with tc.tile_pool(name=f"psB{li}", bufs=2, space="PSUM") as psB, \
                     tc.tile_pool(name=f"psT{li}", bufs=1, space="PSUM") as psT:
                    emitted = [0, 0]  # chunks emitted per set
                    msg_chunks = [[], []]

                    def emit_chunk(s, ci):
                        m = meta[s]
                        ch = m["chunks"][ci]
                        slot0 = ch["tile0"] * 128
                        nslots = ch["ntiles"] * 128
                        it = mep.tile([128, CTmax * 8], I16, tag=f"idx{s}")
                        nc.sync.dma_start(
                            it[:, : nslots // 16],
                            idx_d[s].ap()[:, slot0 // 16 : (slot0 + nslots) // 16],
                        )
                        mt = mp.tile([128, CTmax, H], BF16, tag=f"msg{s}")
                        if MODE == "B":
                            nc.gpsimd.memset(mt[:], 0.0)
                            msg_chunks[s].append((ch["tile0"], mt))
                            return
                        for reg in ch["regions"]:
                            if reg["ntiles"] == 0:
                                continue
                            rel0 = reg["tile0"] - ch["tile0"]
                            stg = sgp.tile([128, RTmax, H], F32, tag=f"stg{s}")
                            mkey = f"stg{s}"
                            if memset_done.get(mkey, 0) < 2:
                                nc.gpsimd.memset(stg[:], 0.0)
                                memset_done[mkey] = memset_done.get(mkey, 0) + 1
                            nidx = reg["ntiles"] * 128
                            c0 = (reg["tile0"] * 128 - slot0) // 16
                            if MODE not in ("A", "B"):
                                nc.gpsimd.dma_gather(
                                    out_ap=stg[:, : reg["ntiles"], :],
                                    in_ap=views[reg["b"]],
                                    idxs_ap=it[:, c0 : c0 + nidx // 16],
                                    num_idxs=nidx,
                                    num_idxs_reg=nidx,
                                    elem_size=H,
                                    elem_step=NB * H,
                                    single_packet=False,
                                )
                            nc.vector.tensor_tensor(
                                out=mt[:, rel0 : rel0 + reg["ntiles"], :],
                                in0=stg[:, : reg["ntiles"], :],
                                in1=ws_sb[s][
                                    :, reg["tile0"] : reg["tile0"] + reg["ntiles"],
                                    None,
                                ].to_broadcast([128, reg["ntiles"], H]),
                                op=mybir.AluOpType.mult,
                            )
                        msg_chunks[s].append((ch["tile0"], mt))

                    def chunk_of_window(s, w):
                        return min(w // NW_CH, len(meta[s]["chunks"]) - 1)

                    for w in range(nw):
                        nodes = min(128, npc - w * 128)
                        cols = slice(w * 128, w * 128 + nodes)
                        for s in (0, 1):
                            need = chunk_of_window(s, min(w + 1, nw - 1)) + 1
                            while emitted[s] < need:
                                emit_chunk(s, emitted[s])
                                emitted[s] += 1
                        ns = []
                        for s in (0, 1):
                            m = meta[s]
                            tw = m["tpw"][w]
                            wm0 = sum(m["tpw"][:w])
                            S_w = s2p.tile([128, tpwmax, 128], BF16, tag=f"S{s}")
                            nc.vector.tensor_tensor(
                                out=S_w[:, :tw, :],
                                in0=nl_sb[s][:, wm0 : wm0 + tw, None].to_broadcast(
                                    [128, tw, 128]
                                ),
                                in1=iota_sb[:, :tw, :],
                                op=mybir.AluOpType.is_equal,
                            )
                            ci = w // NW_CH
                            c_tile0, mt = msg_chunks[s][ci]
                            ps = psB.tile([H, 128], F32, tag=f"n{s}")
                            for j, gt in enumerate(m["gtiles"][w]):
                                nc.tensor.matmul(
                                    ps[:, :nodes],
                                    mt[:, gt - c_tile0, :],
                                    S_w[:, j, :nodes],
                                    start=(j == 0),
                                    stop=(j == tw - 1),
                                )
                            ns_t = nsp.tile([H, 128], BF16, tag=f"ns{s}")
                            nc.scalar.activation(ns_t[:, :nodes], ps[:, :nodes], AF.Copy)
                            ns.append(ns_t)
                        ps3 = psB.tile([H, 128], F32, tag="r")
                        nc.tensor.matmul(
                            ps3[:, :nodes], wl_sb[li][:, 0, :], r_prev[:, cols],
                            start=True, stop=False,
                        )
                        nc.tensor.matmul(
                            ps3[:, :nodes], wl_sb[li][:, 1, :], ns[0][:, :nodes],
                            start=False, stop=False,
                        )
                        nc.tensor.matmul(
                            ps3[:, :nodes], wl_sb[li][:, 2, :], ns[1][:, :nodes],
                            start=False, stop=True,
                        )
                        if li == 0:
                            nc.scalar.activation(
                                r_cur[:, cols], ps3[:, :nodes], AF.Relu,
                                bias=bl_sb[0][:, :1],
                            )
                            pst = psT.tile([128, H], BF16, tag="tr")
                            nc.tensor.transpose(
                                pst[:nodes, :], r_cur[:, cols], ident[:H, :H]
                            )
                            rk_st = stp.tile([128, H], F32, tag="rrow")
                            nc.scalar.activation(
                                rk_st[:nodes, :], pst[:nodes, :], AF.Copy
                            )
                            nc.sync.dma_start(
                                r_loc[1].ap()[w * 128 : w * 128 + nodes, :],
                                rk_st[:nodes, :],
                            )
                        else:
                            # logits: read r0 (rA) and r1 (rB) BEFORE r2
                            # overwrites rA's columns, then add r2's term.
                            ps5 = psT.tile([ncls, 128], F32, tag="lg")
                            nc.tensor.matmul(
                                ps5[:, :nodes], wc_sb[:, 0, :], rA[:, cols],
                                start=True, stop=False,
                            )
                            nc.tensor.matmul(
                                ps5[:, :nodes], wc_sb[:, 1, :], rB[:, cols],
                                start=False, stop=False,
                            )
                            nc.scalar.activation(
                                r_cur[:, cols], ps3[:, :nodes], AF.Relu,
                                bias=bl_sb[1][:, :1],
                            )
                            nc.tensor.matmul(
                                ps5[:, :nodes], wc_sb[:, 2, :], rA[:, cols],
                                start=False, stop=True,
                            )
                            lg_st = stp.tile([ncls, 128], F32, tag="lg")
                            nc.scalar.activation(
                                lg_st[:, :nodes], ps5[:, :nodes], AF.Identity,
                                bias=bc_sb[:, :1],
                            )
                            nc.sync.dma_start(
                                out_d.ap()[:, w * 128 : w * 128 + nodes],
                                lg_st[:, :nodes],
                            )
                if li == 0:
                    nc.gpsimd.collective_compute(
                        "AllGather",
                        mybir.AluOpType.bypass,
                        replica_groups=groups,
                        ins=[r_loc[1].ap().opt()],
                        outs=[r_tab[1].ap().opt()],
                    )

    nc.compile()
    return nc


def prepare(x, edge_index_1, edge_index_2, W_embed, b_embed, W0, b0, W1, b1, Wc, bc,
            ncores=8):
    x = np.asarray(x, np.float32)
    n_nodes, in_dim = x.shape
    npc = n_nodes // ncores
    nw = (npc + 127) // 128
    ipad = ((in_dim + 127) // 128) * 128
    KT = ipad // 128
    ncls = np.asarray(Wc).shape[1]
    H = 64

    e1 = np.asarray(edge_index_1)
    e2 = np.asarray(edge_index_2)
    set0, meta0 = _prep_set(e1[0], e1[1], n_nodes, ncores)
    set1, meta1 = _prep_set(e2[0], e2[1], n_nodes, ncores)

    wemb_p = np.zeros((ipad, H), np.float32)
    wemb_p[:in_dim] = np.asarray(W_embed, np.float32)
    wemb_pack = np.ascontiguousarray(
        wemb_p.reshape(KT, 128, H).transpose(1, 0, 2).reshape(128, KT * H).astype(BF)
    )

    def pack_wl(W):
        W = np.asarray(W, np.float32)
        wr = W[0:64] + W[128:192]
        blocks = np.stack([wr, W[64:128], W[192:256]], axis=1)
        return np.ascontiguousarray(blocks.reshape(H, 3 * H).astype(BF))

    Wc_f = np.asarray(Wc, np.float32)
    wc_pack = np.ascontiguousarray(
        Wc_f.reshape(3, H, ncls).transpose(1, 0, 2).reshape(H, 3 * ncls).astype(BF)
    )

    tpwmax = max(meta0["tpwmax"], meta1["tpwmax"])
    iota = np.broadcast_to(
        np.arange(128, dtype=np.float32), (128, tpwmax, 128)
    ).astype(BF)

    shared = {
        "wemb": wemb_pack,
        "bemb": np.asarray(b_embed, np.float32).reshape(H, 1),
        "wl0": pack_wl(W0),
        "b0": np.asarray(b0, np.float32).reshape(H, 1),
        "wl1": pack_wl(W1),
        "b1": np.asarray(b1, np.float32).reshape(H, 1),
        "wc": wc_pack,
        "bc": np.asarray(bc, np.float32).reshape(ncls, 1),
        "iota": np.ascontiguousarray(iota),
    }
    in_maps = []
    for c in range(ncores):
        xs = x[c * npc : (c + 1) * npc]
        xpad = np.zeros((nw * 128, ipad), np.float32)
        xpad[:npc, :in_dim] = xs
        xpack = (
            xpad.reshape(nw, 128, KT, 128)
            .transpose(3, 0, 2, 1)
            .reshape(128, nw * KT * 128)
            .astype(BF)
        )
        m = dict(shared)
        m["xT"] = np.ascontiguousarray(xpack)
        for s, st in ((0, set0), (1, set1)):
            m[f"idx{s}"] = st[c]["idxw"]
            m[f"ws{s}"] = st[c]["wsg"]
            m[f"nl{s}"] = st[c]["nlw"]
        in_maps.append(m)

    cfg = dict(
        n_nodes=n_nodes, npc=npc, nw=nw, ncores=ncores, ipad=ipad, ncls=ncls,
        meta=[meta0, meta1],
    )
    return cfg, in_maps


_CACHE = {}


def kernel(**inputs):
    ncores = 8
    cfg, in_maps = prepare(**inputs, ncores=ncores)
    key = (cfg["n_nodes"], cfg["npc"], cfg["ncls"],
           cfg["meta"][0]["T"], cfg["meta"][1]["T"],
           cfg["meta"][0]["tpwmax"], cfg["meta"][1]["tpwmax"])
    if key not in _CACHE:
        _CACHE[key] = build_program(cfg)
    nc = _CACHE[key]
    res = run_bass_kernel_spmd(nc, in_maps, core_ids=list(range(ncores)))
    ncls, npc = cfg["ncls"], cfg["npc"]
    out = np.empty((cfg["n_nodes"], ncls), np.float32)
    for c in range(ncores):
        out[c * npc : (c + 1) * npc] = res.results[c]["logitsT"].T
    return out


# revision 19
# speedup vs baseline: 1.5923x; 1.5923x over previous
"""H2GCN forward pass distributed over 8 TRN2 NeuronCores.

Sharding: nodes row-sharded across cores; edges partitioned by src owner so
the segment_sum is core-local; r_prev all-gathered between layers; weights
replicated.

v3 design:
  - Gathers use the dedicated DMAGatherAnt instruction (nc.gpsimd.dma_gather):
    int16 indices wrapped over 16 partitions (replicated x8), 256B rows ->
    f32 table [N, 64]. Index range is handled with 4 parity buckets b = dst&3,
    each gathering through the strided view table[b::4] with idx = dst>>2
    (max 24999 < 2^15). One gather instruction per (8-window chunk, bucket).
  - Per (window, bucket) the slots are padded to whole 128-slot tiles with
    cross-core-uniform tile counts RT[w][b]; pad slots point at view row 0
    with weight 0 and nl=-1.
  - Slot tiles exist in two orders over the same runs: gather order
    (bucket-major within chunk; ws + idx arrays) and window-major order
    (nl array + S tiles). A run is a contiguous identical block in both, so
    window w's j-th S tile pairs with gather tile gtiles[w][j].
  - Messages are converted f32->bf16 by the per-region ws multiply on DVE,
    so PE matmuls stay bf16 (1 cycle/row vs 4 for f32).
  - r^T representations live in two resident SBUF tiles (bf16): layer 0 reads
    rA(=r0) and writes rB(=r1); layer 1 reads rB and writes rA(=r2). Fused
    logits in layer 1 read r0's columns BEFORE the r2 activation overwrites
    them (PSUM accumulation reorder).
  - msg/staging buffers are memset once at first use: stale SBUF bytes must be
    finite because NaN*0 = NaN would poison the matmul.
"""

import os
import sys

sys.path.insert(0, "/opt/trn_rl_repo")

import numpy as np
import ml_dtypes

import concourse.bacc as bacc
import concourse.bass as bass
import concourse.mybir as mybir
import concourse.tile as tile
from concourse.bass_utils import run_bass_kernel_spmd
from concourse.masks import make_identity

F32 = mybir.dt.float32
BF16 = mybir.dt.bfloat16
I16 = mybir.dt.int16
AF = mybir.ActivationFunctionType
BF = ml_dtypes.bfloat16
NB = 4  # parity buckets (dst & 3)
NW_CH = 6  # windows per gather chunk
CH = 7  # phase-A windows per x-chunk load


def _prep_set(src, dst, n_nodes, ncores):
    """Bucketed slot arrays for one edge set.

    Returns (per_core list, meta) where per-core dict has:
      idxw: [128, T*8] int16 wrapped gather indices (gather order)
      wsg:  [128, T] bf16 per-slot weights (gather order)
      nlw:  [128, T] bf16 slot->node-local map (window-major order)
    and meta has RT[w][b], tpw[w], gtiles[w], chunk table, T, tpwmax.
    """
    npc = n_nodes // ncores
    nw = (npc + 127) // 128
    deg = np.bincount(src, minlength=n_nodes)
    wglob = (1.0 / np.maximum(deg, 1.0)).astype(np.float32)

    # per-core edge lists sorted by (local node, bucket)
    cores = []
    cnt = np.zeros((ncores, nw, NB), np.int64)
    for c in range(ncores):
        lo = c * npc
        m = (src >= lo) & (src < lo + npc)
        sl = (src[m] - lo).astype(np.int64)
        d = dst[m].astype(np.int64)
        wv = wglob[src[m]]
        b = d & 3
        w = sl // 128
        order = np.lexsort((sl, b, w))  # sort by (window, bucket, node)
        sl, d, wv, b, w = sl[order], d[order], wv[order], b[order], w[order]
        np.add.at(cnt[c], (w, b), 1)
        cores.append((sl, d, wv, b, w))

    RT = np.maximum(1, -(-cnt.max(axis=0) // 128))  # [nw, NB] tiles
    tpw = RT.sum(axis=1)  # [nw]
    tpwmax = int(tpw.max())

    # chunk table: gather-order tile layout (chunk-major, bucket-major)
    chunks = []
    gtile_of = np.zeros((nw, NB), np.int64)  # first gather tile of (w, b)
    wmtile_of = np.zeros((nw, NB), np.int64)  # first window-major tile
    tile_cursor = 0
    for w0 in range(0, nw, NW_CH):
        wn = min(NW_CH, nw - w0)
        regions = []
        for b in range(NB):
            rt = int(RT[w0 : w0 + wn, b].sum())
            regions.append(dict(b=b, tile0=tile_cursor, ntiles=rt))
            for w in range(w0, w0 + wn):
                gtile_of[w, b] = tile_cursor + int(RT[w0:w, b].sum())
            tile_cursor += rt
        chunks.append(dict(w0=w0, wn=wn, regions=regions))
    T = tile_cursor
    cur = 0
    for w in range(nw):
        for b in range(NB):
            wmtile_of[w, b] = cur
            cur += int(RT[w, b])
    gtiles = [
        [
            int(gtile_of[w, b]) * 1 + r
            for b in range(NB)
            for r in range(int(RT[w, b]))
        ]
        for w in range(nw)
    ]
    for ch in chunks:
        ch["tile0"] = ch["regions"][0]["tile0"]
        ch["ntiles"] = sum(r["ntiles"] for r in ch["regions"])

    per_core = []
    for c in range(ncores):
        sl, d, wv, b, w = cores[c]
        idxg = np.zeros(T * 128, np.int16)  # gather order; pad -> view row 0
        nlw = np.full(T * 128, -1.0, np.float32)  # window-major order
        # rank of each edge within its (w, b) run
        run_start = {}
        pos_g = np.empty(len(sl), np.int64)
        pos_m = np.empty(len(sl), np.int64)
        k = 0
        while k < len(sl):
            w_, b_ = w[k], b[k]
            k2 = k
            while k2 < len(sl) and w[k2] == w_ and b[k2] == b_:
                k2 += 1
            n = k2 - k
            pos_g[k:k2] = gtile_of[w_, b_] * 128 + np.arange(n)
            pos_m[k:k2] = wmtile_of[w_, b_] * 128 + np.arange(n)
            k = k2
        idxg[pos_g] = (d >> 2).astype(np.int16)
        nlw[pos_m] = (sl % 128).astype(np.float32)
        # wrap gather idx: position j -> [j % 16, j // 16], replicated x8
        wrap = idxg.reshape(T * 8, 16).T  # [16, T*8]
        idxw = np.tile(wrap, (8, 1))  # [128, T*8]
        lo_c = c * npc
        invd = np.broadcast_to(
            wglob[lo_c : lo_c + npc], (64, npc)
        ).astype(BF)
        per_core.append(
            dict(
                idxw=np.ascontiguousarray(idxw),
                invd=np.ascontiguousarray(invd),
                nlw=np.ascontiguousarray(nlw.reshape(T, 128).T.astype(BF)),
            )
        )
    meta = dict(
        RT=RT.tolist(),
        tpw=[int(x) for x in tpw],
        tpwmax=tpwmax,
        gtiles=gtiles,
        chunks=chunks,
        T=T,
    )
    return per_core, meta


def build_program(cfg):
    n_nodes = cfg["n_nodes"]
    npc = cfg["npc"]
    nw = cfg["nw"]
    ncores = cfg["ncores"]
    ipad = cfg["ipad"]
    ncls = cfg["ncls"]
    meta = cfg["meta"]  # [set0, set1]
    KT = ipad // 128
    H = 64
    NWC = nw * 128

    T = [meta[s]["T"] for s in (0, 1)]
    tpwmax = max(meta[0]["tpwmax"], meta[1]["tpwmax"])
    CTmax = max(
        max(ch["ntiles"] for ch in meta[s]["chunks"]) for s in (0, 1)
    )
    RTmax = max(
        max(r["ntiles"] for ch in meta[s]["chunks"] for r in ch["regions"])
        for s in (0, 1)
    )

    MODE = os.environ.get("KBISECT", "full")
    nc = bacc.Bacc(
        "TRN2",
        target_bir_lowering=False,
        debug=False,
        enable_asserts=False,
        num_devices=ncores,
    )

    # --- DRAM I/O ---
    xT = nc.dram_tensor("xT", [128, nw * KT * 128], BF16, kind="ExternalInput")
    wemb = nc.dram_tensor("wemb", [128, KT * H], BF16, kind="ExternalInput")
    bemb = nc.dram_tensor("bemb", [H, 1], F32, kind="ExternalInput")
    wl = [
        nc.dram_tensor(f"wl{i}", [H, 3 * H], BF16, kind="ExternalInput")
        for i in (0, 1)
    ]
    bl = [nc.dram_tensor(f"b{i}", [H, 1], F32, kind="ExternalInput") for i in (0, 1)]
    wc = nc.dram_tensor("wc", [H, 3 * ncls], BF16, kind="ExternalInput")
    bc = nc.dram_tensor("bc", [ncls, 1], F32, kind="ExternalInput")
    iota = nc.dram_tensor("iota", [128, 128 * 2 * tpwmax], BF16, kind="ExternalInput")
    idx_d = [
        nc.dram_tensor(f"idx{s}", [128, T[s] * 8], I16, kind="ExternalInput")
        for s in (0, 1)
    ]
    invd_d = [
        nc.dram_tensor(f"invd{s}", [64, npc], BF16, kind="ExternalInput")
        for s in (0, 1)
    ]
    nl_d = [
        nc.dram_tensor(f"nl{s}", [128, T[s]], BF16, kind="ExternalInput")
        for s in (0, 1)
    ]
    out_d = nc.dram_tensor("logitsT", [ncls, npc], F32, kind="ExternalOutput")

    r_loc = [nc.dram_tensor(f"r{k}_loc", [npc, H], F32) for k in (0, 1)]
    r_tab = [
        nc.dram_tensor(f"r{k}_tab", [n_nodes, H], F32, addr_space="Shared")
        for k in (0, 1)
    ]

    groups = [list(range(ncores))]

    with tile.TileContext(nc) as tc:
        with (
            tc.tile_pool(name="const", bufs=1) as cp,
            tc.tile_pool(name="io", bufs=2) as iop,
            tc.tile_pool(name="msg", bufs=2) as mp,
            tc.tile_pool(name="stg", bufs=2) as sgp,
            tc.tile_pool(name="meta", bufs=2) as mep,
            tc.tile_pool(name="s2", bufs=2) as s2p,
            tc.tile_pool(name="ns", bufs=2) as nsp,
            tc.tile_pool(name="st", bufs=3) as stp,
        ):
            ident = cp.tile([128, 128], BF16, tag="ident")
            make_identity(nc, ident[:])
            iota_sb = cp.tile([128, 128, 2 * tpwmax], BF16, tag="iota")
            nc.sync.dma_start(
                iota_sb[:], iota.ap().rearrange("p (j t) -> p j t", t=2 * tpwmax)
            )
            wemb_sb = cp.tile([128, KT, H], BF16, tag="wemb")
            nc.sync.dma_start(
                wemb_sb[:], wemb.ap().rearrange("p (k h) -> p k h", k=KT)
            )
            wl_sb = []
            for i in (0, 1):
                t = cp.tile([H, 3, H], BF16, tag=f"wl{i}")
                nc.sync.dma_start(t[:], wl[i].ap().rearrange("p (s h) -> p s h", s=3))
                wl_sb.append(t)
            wc_sb = cp.tile([H, 3, ncls], BF16, tag="wc")
            nc.sync.dma_start(wc_sb[:], wc.ap().rearrange("p (s h) -> p s h", s=3))
            bemb_sb = cp.tile([H, 1], F32, tag="bemb")
            nc.sync.dma_start(bemb_sb[:], bemb[:])
            bl_sb = []
            for i in (0, 1):
                t = cp.tile([H, 1], F32, tag=f"bl{i}")
                nc.sync.dma_start(t[:], bl[i][:])
                bl_sb.append(t)
            bc_sb = cp.tile([ncls, 1], F32, tag="bc")
            nc.sync.dma_start(bc_sb[:], bc[:])
            nl_sb = []
            for s in (0, 1):
                t = cp.tile([128, T[s]], BF16, tag=f"nl{s}", name=f"nl{s}")
                nc.sync.dma_start(t[:], nl_d[s][:])
                nl_sb.append(t)

            # two resident transposed representations [64, NWC] bf16
            rA = cp.tile([H, NWC], BF16, tag="rA")  # r0, later overwritten by r2
            rB = cp.tile([H, NWC], BF16, tag="rB")  # r1

            # ---------------- Phase A: r0 = relu(x @ Wemb + b) ----------------
            with tc.tile_pool(name="psA", bufs=2, space="PSUM") as psA:
                for clo in range(0, nw, CH):
                    chn = min(CH, nw - clo)
                    xw = iop.tile([128, CH, KT, 128], BF16, tag="xw")
                    nc.sync.dma_start(
                        xw[:, :chn],
                        xT.ap().rearrange("p (w k n) -> p w k n", k=KT, n=128)[
                            :, clo : clo + chn
                        ],
                    )
                    for wi in range(chn):
                        w = clo + wi
                        nodes = min(128, npc - w * 128)
                        cols = slice(w * 128, w * 128 + nodes)
                        ps = psA.tile([H, 128], F32, tag="e")
                        for k in range(KT):
                            nc.tensor.matmul(
                                ps[:, :nodes],
                                wemb_sb[:, k, :],
                                xw[:, wi, k, :nodes],
                                start=(k == 0),
                                stop=(k == KT - 1),
                            )
                        nc.scalar.activation(
                            rA[:, cols], ps[:, :nodes], AF.Relu, bias=bemb_sb[:, :1]
                        )
                        pst = psA.tile([128, H], BF16, tag="tr")
                        nc.tensor.transpose(pst[:nodes, :], rA[:, cols], ident[:H, :H])
                        r0_st = stp.tile([128, H], F32, tag="rrow")
                        nc.scalar.activation(r0_st[:nodes, :], pst[:nodes, :], AF.Copy)
                        nc.sync.dma_start(
                            r_loc[0].ap()[w * 128 : w * 128 + nodes, :],
                            r0_st[:nodes, :],
                        )

            nc.gpsimd.collective_compute(
                "AllGather",
                mybir.AluOpType.bypass,
                replica_groups=groups,
                ins=[r_loc[0].ap().opt()],
                outs=[r_tab[0].ap().opt()],
            )

            # ---------------- Layers ----------------
            if MODE == "A":
                zt = cp.tile([ncls, 128], F32, tag="zero")
                nc.gpsimd.memset(zt[:], 0.0)
                for w in range(nw):
                    nodes = min(128, npc - w * 128)
                    nc.sync.dma_start(
                        out_d.ap()[:, w * 128 : w * 128 + nodes], zt[:, :nodes]
                    )
            memset_done = {}
            for li in ((), (0, 1))[MODE != "A"]:
                table = r_tab[li]
                # strided per-bucket views: rows b::4, stride 4*64 elems
                views = [table.ap()[b::4, :] for b in range(NB)]
                r_prev = rA if li == 0 else rB
                r_cur = rB if li == 0 else rA
                with tc.tile_pool(name=f"psB{li}", bufs=2, space="PSUM") as psB, \
                     tc.tile_pool(name=f"psT{li}", bufs=1, space="PSUM") as psT:
                    emitted = [0, 0]  # chunks emitted per set
                    msg_chunks = [[], []]
                    S_pair = [None, None]
                    S_base = [0, 0]

                    def emit_chunk(s, ci):
                        m = meta[s]
                        ch = m["chunks"][ci]
                        slot0 = ch["tile0"] * 128
                        nslots = ch["ntiles"] * 128
                        it = mep.tile([128, CTmax * 8], I16, tag=f"idx{s}")
                        nc.sync.dma_start(
                            it[:, : nslots // 16],
                            idx_d[s].ap()[:, slot0 // 16 : (slot0 + nslots) // 16],
                        )
                        mt = mp.tile([128, CTmax, H], BF16, tag=f"msg{s}")
                        if MODE == "B":
                            nc.gpsimd.memset(mt[:], 0.0)
                            msg_chunks[s].append((ch["tile0"], mt))
                            return
                        for reg in ch["regions"]:
                            if reg["ntiles"] == 0:
                                continue
                            rel0 = reg["tile0"] - ch["tile0"]
                            stg = sgp.tile([128, RTmax, H], F32, tag=f"stg{s}")
                            mkey = f"stg{s}"
                            if memset_done.get(mkey, 0) < 2:
                                nc.gpsimd.memset(stg[:], 0.0)
                                memset_done[mkey] = memset_done.get(mkey, 0) + 1
                            nidx = reg["ntiles"] * 128
                            c0 = (reg["tile0"] * 128 - slot0) // 16
                            if MODE not in ("A", "B"):
                                nc.gpsimd.dma_gather(
                                    out_ap=stg[:, : reg["ntiles"], :],
                                    in_ap=views[reg["b"]],
                                    idxs_ap=it[:, c0 : c0 + nidx // 16],
                                    num_idxs=nidx,
                                    num_idxs_reg=nidx,
                                    elem_size=H,
                                    elem_step=NB * H,
                                    single_packet=False,
                                )
                            nc.scalar.activation(
                                mt[:, rel0 : rel0 + reg["ntiles"], :],
                                stg[:, : reg["ntiles"], :],
                                AF.Copy,
                            )
                        msg_chunks[s].append((ch["tile0"], mt))

                    def chunk_of_window(s, w):
                        return min(w // NW_CH, len(meta[s]["chunks"]) - 1)

                    for w in range(nw):
                        nodes = min(128, npc - w * 128)
                        cols = slice(w * 128, w * 128 + nodes)
                        for s in (0, 1):
                            need = chunk_of_window(s, min(w + 1, nw - 1)) + 1
                            while emitted[s] < need:
                                emit_chunk(s, emitted[s])
                                emitted[s] += 1
                        ns = []
                        invt = []
                        for s in (0, 1):
                            tt = mep.tile([H, 128], BF16, tag=f"invd{s}")
                            nc.sync.dma_start(
                                tt[:, :nodes], invd_d[s].ap()[:, cols]
                            )
                            invt.append(tt)
                        for s in (0, 1):
                            m = meta[s]
                            tw = m["tpw"][w]
                            wm0 = sum(m["tpw"][:w])
                            if w % 2 == 0:
                                tww = tw + (m["tpw"][w + 1] if w + 1 < nw else 0)
                                S_p = s2p.tile(
                                    [128, 128, 2 * tpwmax], BF16, tag=f"S{s}"
                                )
                                nc.vector.tensor_tensor(
                                    out=S_p[:, :, :tww],
                                    in0=nl_sb[s][
                                        :, None, wm0 : wm0 + tww
                                    ].to_broadcast([128, 128, tww]),
                                    in1=iota_sb[:, :, :tww],
                                    op=mybir.AluOpType.is_equal,
                                )
                                S_pair[s] = S_p
                                S_base[s] = wm0
                            S_w = S_pair[s]
                            off = wm0 - S_base[s]
                            ci = w // NW_CH
                            c_tile0, mt = msg_chunks[s][ci]
                            ps = psB.tile([H, 128], F32, tag=f"n{s}")
                            for j, gt in enumerate(m["gtiles"][w]):
                                nc.tensor.matmul(
                                    ps[:, :nodes],
                                    mt[:, gt - c_tile0, :],
                                    S_w[:, :nodes, off + j],
                                    start=(j == 0),
                                    stop=(j == tw - 1),
                                )
                            ns_t = nsp.tile([H, 128], BF16, tag=f"ns{s}")
                            nc.vector.tensor_tensor(
                                out=ns_t[:, :nodes],
                                in0=ps[:, :nodes],
                                in1=invt[s][:, :nodes],
                                op=mybir.AluOpType.mult,
                            )
                            ns.append(ns_t)
                        ps3 = psB.tile([H, 128], F32, tag="r")
                        nc.tensor.matmul(
                            ps3[:, :nodes], wl_sb[li][:, 0, :], r_prev[:, cols],
                            start=True, stop=False,
                        )
                        nc.tensor.matmul(
                            ps3[:, :nodes], wl_sb[li][:, 1, :], ns[0][:, :nodes],
                            start=False, stop=False,
                        )
                        nc.tensor.matmul(
                            ps3[:, :nodes], wl_sb[li][:, 2, :], ns[1][:, :nodes],
                            start=False, stop=True,
                        )
                        if li == 0:
                            nc.scalar.activation(
                                r_cur[:, cols], ps3[:, :nodes], AF.Relu,
                                bias=bl_sb[0][:, :1],
                            )
                            pst = psT.tile([128, H], BF16, tag="tr")
                            nc.tensor.transpose(
                                pst[:nodes, :], r_cur[:, cols], ident[:H, :H]
                            )
                            rk_st = stp.tile([128, H], F32, tag="rrow")
                            nc.scalar.activation(
                                rk_st[:nodes, :], pst[:nodes, :], AF.Copy
                            )
                            nc.sync.dma_start(
                                r_loc[1].ap()[w * 128 : w * 128 + nodes, :],
                                rk_st[:nodes, :],
                            )
                        else:
                            # logits: read r0 (rA) and r1 (rB) BEFORE r2
                            # overwrites rA's columns, then add r2's term.
                            ps5 = psT.tile([ncls, 128], F32, tag="lg")
                            nc.tensor.matmul(
                                ps5[:, :nodes], wc_sb[:, 0, :], rA[:, cols],
                                start=True, stop=False,
                            )
                            nc.tensor.matmul(
                                ps5[:, :nodes], wc_sb[:, 1, :], rB[:, cols],
                                start=False, stop=False,
                            )
                            nc.scalar.activation(
                                r_cur[:, cols], ps3[:, :nodes], AF.Relu,
                                bias=bl_sb[1][:, :1],
                            )
                            nc.tensor.matmul(
                                ps5[:, :nodes], wc_sb[:, 2, :], rA[:, cols],
                                start=False, stop=True,
                            )
                            lg_st = stp.tile([ncls, 128], F32, tag="lg")
                            nc.scalar.activation(
                                lg_st[:, :nodes], ps5[:, :nodes], AF.Identity,
                                bias=bc_sb[:, :1],
                            )
                            nc.sync.dma_start(
                                out_d.ap()[:, w * 128 : w * 128 + nodes],
                                lg_st[:, :nodes],
                            )
                if li == 0:
                    nc.gpsimd.collective_compute(
                        "AllGather",
                        mybir.AluOpType.bypass,
                        replica_groups=groups,
                        ins=[r_loc[1].ap().opt()],
                        outs=[r_tab[1].ap().opt()],
                    )

    nc.compile()
    return nc


def prepare(x, edge_index_1, edge_index_2, W_embed, b_embed, W0, b0, W1, b1, Wc, bc,
            ncores=8):
    x = np.asarray(x, np.float32)
    n_nodes, in_dim = x.shape
    npc = n_nodes // ncores
    nw = (npc + 127) // 128
    ipad = ((in_dim + 127) // 128) * 128
    KT = ipad // 128
    ncls = np.asarray(Wc).shape[1]
    H = 64

    e1 = np.asarray(edge_index_1)
    e2 = np.asarray(edge_index_2)
    set0, meta0 = _prep_set(e1[0], e1[1], n_nodes, ncores)
    set1, meta1 = _prep_set(e2[0], e2[1], n_nodes, ncores)

    wemb_p = np.zeros((ipad, H), np.float32)
    wemb_p[:in_dim] = np.asarray(W_embed, np.float32)
    wemb_pack = np.ascontiguousarray(
        wemb_p.reshape(KT, 128, H).transpose(1, 0, 2).reshape(128, KT * H).astype(BF)
    )

    def pack_wl(W):
        W = np.asarray(W, np.float32)
        wr = W[0:64] + W[128:192]
        blocks = np.stack([wr, W[64:128], W[192:256]], axis=1)
        return np.ascontiguousarray(blocks.reshape(H, 3 * H).astype(BF))

    Wc_f = np.asarray(Wc, np.float32)
    wc_pack = np.ascontiguousarray(
        Wc_f.reshape(3, H, ncls).transpose(1, 0, 2).reshape(H, 3 * ncls).astype(BF)
    )

    tpwmax = max(meta0["tpwmax"], meta1["tpwmax"])
    iota = np.broadcast_to(
        np.arange(128, dtype=np.float32)[None, :, None],
        (128, 128, 2 * tpwmax),
    ).astype(BF).reshape(128, 128 * 2 * tpwmax)

    shared = {
        "wemb": wemb_pack,
        "bemb": np.asarray(b_embed, np.float32).reshape(H, 1),
        "wl0": pack_wl(W0),
        "b0": np.asarray(b0, np.float32).reshape(H, 1),
        "wl1": pack_wl(W1),
        "b1": np.asarray(b1, np.float32).reshape(H, 1),
        "wc": wc_pack,
        "bc": np.asarray(bc, np.float32).reshape(ncls, 1),
        "iota": np.ascontiguousarray(iota),
    }
    in_maps = []
    for c in range(ncores):
        xs = x[c * npc : (c + 1) * npc]
        xpad = np.zeros((nw * 128, ipad), np.float32)
        xpad[:npc, :in_dim] = xs
        xpack = (
            xpad.reshape(nw, 128, KT, 128)
            .transpose(3, 0, 2, 1)
            .reshape(128, nw * KT * 128)
            .astype(BF)
        )
        m = dict(shared)
        m["xT"] = np.ascontiguousarray(xpack)
        for s, st in ((0, set0), (1, set1)):
            m[f"idx{s}"] = st[c]["idxw"]
            m[f"invd{s}"] = st[c]["invd"]
            m[f"nl{s}"] = st[c]["nlw"]
        in_maps.append(m)

    cfg = dict(
        n_nodes=n_nodes, npc=npc, nw=nw, ncores=ncores, ipad=ipad, ncls=ncls,
        meta=[meta0, meta1],
    )
    return cfg, in_maps


_CACHE = {}


def kernel(**inputs):
    ncores = 8
    cfg, in_maps = prepare(**inputs, ncores=ncores)
    key = (cfg["n_nodes"], cfg["npc"], cfg["ncls"],
           cfg["meta"][0]["T"], cfg["meta"][1]["T"],
           cfg["meta"][0]["tpwmax"], cfg["meta"][1]["tpwmax"])
    if key not in _CACHE:
        _CACHE[key] = build_program(cfg)
    nc = _CACHE[key]
    res = run_bass_kernel_spmd(nc, in_maps, core_ids=list(range(ncores)))
    ncls, npc = cfg["ncls"], cfg["npc"]
    out = np.empty((cfg["n_nodes"], ncls), np.float32)
    for c in range(ncores):
        out[c * npc : (c + 1) * npc] = res.results[c]["logitsT"].T
    return out
